# revision 1
# baseline (speedup 1.0000x reference)
"""Distributed multi-head attention kernel for 8 Trainium2 NeuronCores.

Problem: x[4,2048,1024] -> qkv proj -> 16-head attention (add_zero_attn)
         -> out proj + bias -> [4,2048,1024]

Sharding: 8 cores = 4 batches x 2 query-halves. Each core computes the
full K/V for its batch (KV projection duplicated across the pair, ~4GFLOP,
far cheaper than any 2-rank collective on this fabric) and attention +
output projection for its own 1024 queries. Zero collectives; host
reassembles by concatenation only.

add_zero_attn appends a zero key & value token: the value row is zero so it
only adds +1 to each softmax denominator. We therefore never materialize it;
denominators come from a ones-column appended to V (column 64 of each head's
v tile) and get +1 before the reciprocal.

Schedule: the attention inner loop is ScalarE-bound (exp of 33.5M scores at
1 elem/cycle/lane), so the projections for head-pair g+1 are chopped into
small work quanta and drained inside head-pair g's attention loop, filling
the PE gaps while ScalarE streams exp. Dedicated psum pools per purpose
(S-scores / o-accumulate / everything-else) keep the pool FIFOs from
cross-serializing the phases. o-matmuls lag the exp stream by 2 iterations
so the PE queue head never waits on ScalarE. W_out prefetches during early
attention; out-proj for the first query half overlaps the last pair's
attention, leaving only the second half as tail.
"""

import sys

sys.path.insert(0, "/opt/trn_rl_repo")

from contextlib import ExitStack

import numpy as np

import concourse.bass as bass
import concourse.tile as tile
from concourse import bacc, mybir
from concourse.masks import make_identity

P = 128
B, N, D = 4, 2048, 1024
H, DH = 16, 64
INNER = H * DH  # 1024
SCALE = DH ** -0.5
NQ = N // 2     # queries per core
NCORES = 8

F32 = mybir.dt.float32
BF16 = mybir.dt.bfloat16

DC = D // P         # 8 chunks of the model dim
IC = INNER // P     # 8 chunks of the inner dim
TK = N // P         # 16 kpos chunks
NG = H // 2         # 8 head pairs
QB = NQ // 512      # 2 query blocks of 512


def _build_body(ctx: ExitStack, tc, out_ext, x_ext, wqkv_ext, wout_ext, bout_ext):
    nc = tc.nc

    dram_pool = ctx.enter_context(tc.tile_pool(name="dram", bufs=1, space="DRAM"))
    ddram = dram_pool.tile([NG, QB, 2, 512], F32, tag="ddram")

    consts = ctx.enter_context(tc.tile_pool(name="consts", bufs=1))
    xstage = ctx.enter_context(tc.tile_pool(name="xstage", bufs=3))
    wstage = ctx.enter_context(tc.tile_pool(name="wstage", bufs=3))
    wqk_pool = ctx.enter_context(tc.tile_pool(name="wqk", bufs=3))
    wv_pool = ctx.enter_context(tc.tile_pool(name="wv", bufs=1))
    persist = ctx.enter_context(tc.tile_pool(name="persist", bufs=1))
    aw_pool = ctx.enter_context(tc.tile_pool(name="attnw", bufs=3))
    d_pool = ctx.enter_context(tc.tile_pool(name="dsb", bufs=2))
    bcs_pool = ctx.enter_context(tc.tile_pool(name="bcsb", bufs=2))
    outst = ctx.enter_context(tc.tile_pool(name="outst", bufs=2))

    # psum: 2x[128,1024] scores (banks 0-3), 2x[128,512] o-accum (4-5),
    # 2x[128,512] universal for proj/transpose/outproj (6-7)
    psum_s = ctx.enter_context(tc.tile_pool(name="psum_s", bufs=2, space="PSUM"))
    psum_o = ctx.enter_context(tc.tile_pool(name="psum_o", bufs=2, space="PSUM"))
    psum_univ = ctx.enter_context(tc.tile_pool(name="psum_univ", bufs=2, space="PSUM"))

    # ---- persistent SBUF arrays ----
    # xT split into 4 independent 512-token slabs: all consumers read
    # within one slab, so transposes never false-conflict with proj reads
    xT_slabs = [persist.tile([P, DC, 512], BF16, tag=f"xT{s}", name=f"xT_{s}")
                for s in range(4)]

    def xT(c, lo, width):
        s, off = lo // 512, lo % 512
        assert off + width <= 512
        return xT_slabs[s][:, c, off:off + width]

    qT = persist.tile([P, IC, NQ], BF16, tag="qT")
    kT = persist.tile([P, IC, N], BF16, tag="kT")
    v_sb = persist.tile([P, TK, H, DH + 1], BF16, tag="v")   # col DH is ones
    oT = persist.tile([P, IC, NQ], BF16, tag="oT")
    wout = persist.tile([P, IC, D], BF16, tag="wout")

    identity = consts.tile([P, P], F32, tag="ident")
    make_identity(nc, identity)
    ones_lhsT = consts.tile([1, P], BF16, tag="ones")
    nc.vector.memset(ones_lhsT, 1.0)
    ones_f32 = consts.tile([1, P], F32, tag="onesf")
    nc.vector.memset(ones_f32, 1.0)
    bout_bf = consts.tile([1, D], BF16, tag="bout")
    bout_f32 = consts.tile([1, D], F32, tag="boutf")
    nc.gpsimd.dma_start(bout_f32, bout_ext)
    nc.vector.tensor_copy(bout_bf, bout_f32)

    # ones column of v (written once; v evictions fill the rest)
    for t in range(TK):
        nc.vector.memset(v_sb[:, t, :, DH:DH + 1], 1.0)

    # ---- x: load then transpose 128x128 blocks on the PE (identity
    # matmul) into the score psum pool (idle during the prologue);
    # eviction casts f32 psum -> bf16 xT ----
    def transpose_chunk(r):
        x_f = xstage.tile([P, D], F32, tag="xf", name=f"xf_{r}")
        # split across four DMA queue slots to cut arrival latency
        for s in range(4):
            nc.sync.dma_start(x_f[:, s * (D // 4):(s + 1) * (D // 4)],
                              x_ext[r * P:(r + 1) * P,
                                    s * (D // 4):(s + 1) * (D // 4)])
        tp = psum_s.tile([P, 1024], F32, tag="st", name=f"tp_{r}")
        for c in range(DC):
            nc.tensor.transpose(tp[:, c * P:(c + 1) * P],
                                x_f[:, c * P:(c + 1) * P], identity)
        # evict on ScalarE: it sits idle until the first exp, while the
        # Vector engine is busy with weight casts during the prologue
        nc.scalar.copy(
            xT_slabs[r // 4][:, :, (r % 4) * P:(r % 4 + 1) * P],
            tp.rearrange("p (c f) -> p c f", c=DC),
        )

    wqkv_view = wqkv_ext.rearrange("(c p) f -> p c f", p=P)   # [128, DC, 3072]
    wout_view = wout_ext.rearrange("(c p) f -> p c f", p=P)   # [128, IC, 1024]

    # ---------------- work-quantum builders ----------------
    # Each quantum is a closure doing ~400-900ns of engine work. They are
    # drained a few per attention iteration so the PE never idles while
    # ScalarE streams exp, and ScalarE starts as early as possible.

    def gen_pair_qk(g):
        """Work items computing qT[:,g] and kT[:,g]."""
        items = []
        holder = {}

        def load_w(m, key):
            def run():
                w_f = wstage.tile([P, DC, P], F32, tag="wf", name=f"wf_{key}")
                nc.sync.dma_start(w_f, wqkv_view[:, :, m * P:(m + 1) * P])
                w_b = wqk_pool.tile([P, DC, P], BF16, tag="wqk", name=f"wb_{key}")
                nc.vector.tensor_copy(w_b, w_f)
                holder[key] = w_b
            return run

        # qT column block g: two 512-query halves
        items.append(load_w(g, f"q{g}"))
        for j in range(QB):
            def qa(j=j):
                ps = psum_univ.tile([P, 512], F32, tag="u512", name=f"pq_{g}_{j}")
                holder[f"pq{j}"] = ps
                for c in range(4):
                    nc.tensor.matmul(ps, holder[f"q{g}"][:, c, :],
                                     xT(c, j * 512, 512),
                                     start=(c == 0), stop=False)
            def qb_(j=j):
                ps = holder[f"pq{j}"]
                for c in range(4, DC):
                    nc.tensor.matmul(ps, holder[f"q{g}"][:, c, :],
                                     xT(c, j * 512, 512),
                                     start=False, stop=(c == DC - 1))
            def qe(j=j):
                nc.vector.tensor_copy(qT[:, g, j * 512:(j + 1) * 512],
                                      holder[f"pq{j}"])
            items += [qa, qb_, qe]

        # kT column block g: four 512-token blocks
        items.append(load_w(8 + g, f"k{g}"))
        for nj in range(4):
            def ka(nj=nj):
                ps = psum_univ.tile([P, 512], F32, tag="u512", name=f"pk_{g}_{nj}")
                holder[f"pk{nj}"] = ps
                for c in range(4):
                    nc.tensor.matmul(ps, holder[f"k{g}"][:, c, :],
                                     xT(c, nj * 512, 512),
                                     start=(c == 0), stop=False)
            def kb(nj=nj):
                ps = holder[f"pk{nj}"]
                for c in range(4, DC):
                    nc.tensor.matmul(ps, holder[f"k{g}"][:, c, :],
                                     xT(c, nj * 512, 512),
                                     start=False, stop=(c == DC - 1))
            def ke(nj=nj):
                nc.vector.tensor_copy(kT[:, g, nj * 512:(nj + 1) * 512],
                                      holder[f"pk{nj}"])
            items += [ka, kb, ke]
        return items

    def gen_v_half(nh):
        """Work items computing v for heads nh*8 .. nh*8+7 (pairs 4nh..4nh+3).

        v needs kpos on partitions, so x^T chunks are the stationary operand
        and the 8-head weight slab [128, DC, 512] is the moving one (N=512)."""
        items = []
        holder = {}

        def load_wv(blk):
            def run():
                if blk == 0:
                    holder["wvh"] = wv_pool.tile([P, DC, 512], BF16, tag="wvh",
                                                 name=f"wvh_{nh}")
                m = 16 + nh * 4 + blk
                w_f = wstage.tile([P, DC, P], F32, tag="wf", name=f"wf_v_{m}")
                nc.sync.dma_start(w_f, wqkv_view[:, :, m * P:(m + 1) * P])
                nc.vector.tensor_copy(
                    holder["wvh"][:, :, blk * P:(blk + 1) * P], w_f)
            return run

        for blk in range(4):
            items.append(load_wv(blk))
        for t in range(TK):
            def va(t=t):
                ps = psum_univ.tile([P, 512], F32, tag="u512",
                                    name=f"pv_{t}_{nh}")
                holder[f"pv{t}"] = ps
                for c in range(4):
                    nc.tensor.matmul(ps, xT(c, t * P, P),
                                     holder["wvh"][:, c, :],
                                     start=(c == 0), stop=False)
            def vb(t=t):
                ps = holder[f"pv{t}"]
                for c in range(4, DC):
                    nc.tensor.matmul(ps, xT(c, t * P, P),
                                     holder["wvh"][:, c, :],
                                     start=False, stop=(c == DC - 1))
            def ve(t=t):
                nc.vector.tensor_copy(
                    v_sb[:, t, nh * 8:(nh + 1) * 8, 0:DH],
                    holder[f"pv{t}"].rearrange("p (h d) -> p h d", h=8),
                )
            items += [va, vb, ve]
        return items

    def gen_wout_load():
        items = []
        for m in range(DC):
            def run(m=m):
                w_f = wstage.tile([P, IC, P], F32, tag="wf", name=f"wf_o_{m}")
                nc.sync.dma_start(w_f, wout_view[:, :, m * P:(m + 1) * P])
                nc.vector.tensor_copy(wout[:, :, m * P:(m + 1) * P], w_f)
            items.append(run)
        return items

    def gen_outproj(t_lo, t_hi):
        """Output projection + bias for token chunks [t_lo, t_hi)."""
        items = []
        holder = {}
        for t in range(t_lo, t_hi):
            for fh in range(2):
                def oa(t=t, fh=fh):
                    ps = psum_univ.tile([P, 512], F32, tag="u512",
                                        name=f"pout_{t}_{fh}")
                    holder[(t, fh)] = ps
                    for c in range(4):
                        nc.tensor.matmul(ps, oT[:, c, t * P:(t + 1) * P],
                                         wout[:, c, fh * 512:(fh + 1) * 512],
                                         start=(c == 0), stop=False)
                def ob(t=t, fh=fh):
                    ps = holder[(t, fh)]
                    for c in range(4, IC):
                        nc.tensor.matmul(ps, oT[:, c, t * P:(t + 1) * P],
                                         wout[:, c, fh * 512:(fh + 1) * 512],
                                         start=False, stop=False)
                    nc.tensor.matmul(ps, ones_lhsT,
                                     bout_bf[:, fh * 512:(fh + 1) * 512],
                                     start=False, stop=True)
                def oe(t=t, fh=fh):
                    o_sb = outst.tile([P, 512], F32, tag="osb",
                                      name=f"osb_{t}_{fh}")
                    nc.vector.tensor_copy(o_sb, holder[(t, fh)])
                    nc.gpsimd.dma_start(
                        out_ext[t * P:(t + 1) * P, fh * 512:(fh + 1) * 512],
                        o_sb)
                items += [oa, ob, oe]
        return items

    # ---------------- attention ----------------
    pending = []
    d_store = {}

    def flush_normalize():
        # drain all finished groups: broadcast 1/(rowsum+1) over the 64
        # partitions of each head and normalize oT in place
        while pending:
            gg, gqb = pending.pop(0)
            bc = bcs_pool.tile([P, 512], F32, tag="bcs", name=f"bc_{gg}_{gqb}")
            for h01 in range(2):
                a = ddram[gg, gqb, h01]
                bcast_src = bass.AP(tensor=a.tensor, offset=a.offset,
                                    ap=[[0, 64]] + list(a.ap))
                nc.gpsimd.dma_start(bc[h01 * 64:(h01 + 1) * 64, :], bcast_src)
            nc.vector.reciprocal_approx_fast(bc, bc)
            nc.vector.tensor_mul(
                oT[:, gg, gqb * 512:(gqb + 1) * 512],
                oT[:, gg, gqb * 512:(gqb + 1) * 512],
                bc,
            )

    N_IT = QB * TK

    def spread(items, it_lo, it_hi):
        """Distribute work items evenly across iterations [it_lo, it_hi)."""
        sched = [[] for _ in range(N_IT)]
        span = it_hi - it_lo
        for i in range(span):
            lo = len(items) * i // span
            hi = len(items) * (i + 1) // span
            sched[it_lo + i].extend(items[lo:hi])
        return sched

    def merge(*scheds):
        out = [[] for _ in range(N_IT)]
        for s in scheds:
            for i, lst in enumerate(s):
                out[i].extend(lst)
        return out

    # o-matmuls for the last two kpos chunks of a unit (and its eviction)
    # are deferred into the next unit's first iterations so the next unit's
    # S/exp stream starts without an o-tail bubble on ScalarE
    tail_work = []

    def attention(g, sched):
        """Attention for head pair g; processes kpos chunks in pairs
        (two S-pairs back to back amortize the PE row-tile mode switch)
        and drains sched[it] along the way."""

        def drain(it):
            for w in sched[it]:
                w()

        for qb in range(QB):
            o_ps = [None, None]
            aw_ring = {}
            for kc in range(TK):
                if kc == 8:
                    flush_normalize()
                st = psum_s.tile([P, 1024], F32, tag="st",
                                 name=f"st_{g}_{qb}_{kc}")
                for h01 in range(2):
                    lo = h01 * 64
                    nc.tensor.matmul(
                        st[:, h01 * 512:(h01 + 1) * 512],
                        kT[lo:lo + 64, g, kc * P:(kc + 1) * P],
                        qT[lo:lo + 64, g, qb * 512:(qb + 1) * 512],
                        start=True, stop=True,
                    )
                aw = aw_pool.tile([P, 1024], BF16, tag="aw",
                                  name=f"aw_{g}_{qb}_{kc}")
                nc.scalar.activation(
                    aw, st, mybir.ActivationFunctionType.Exp, scale=SCALE,
                )
                aw_ring[kc] = aw
                drain(qb * TK + kc)
                if kc == 0 and tail_work:
                    tail_work[0]()
                elif kc == 1 and tail_work:
                    tail_work[1]()
                    tail_work.clear()
                if kc >= 2:
                    emit_o(g, qb, kc - 2, o_ps, aw_ring.pop(kc - 2))
            tail_work[:] = [make_tail(g, qb, TK - 2, o_ps, aw_ring.pop(TK - 2),
                                      evict=False),
                            make_tail(g, qb, TK - 1, o_ps, aw_ring.pop(TK - 1),
                                      evict=True)]

    def make_tail(g, qb, kc, o_ps, aw, evict):
        def run():
            emit_o(g, qb, kc, o_ps, aw)
            if evict:
                evict_unit(g, qb, o_ps)
        return run

    def evict_unit(g, qb, o_ps):
        # evict: D rows (+1 for the zero-attn token) and unnormalized oT
        for h01 in range(2):
            d_sb = d_pool.tile([1, 512], F32, tag="dsb",
                               name=f"d_{g}_{qb}_{h01}")
            nc.vector.tensor_scalar_add(d_sb, o_ps[h01][DH:DH + 1, :], 1.0)
            nc.gpsimd.dma_start(ddram[g, qb, h01], d_sb)
            d_store[(g, qb, h01)] = d_sb
            nc.vector.tensor_copy(
                oT[h01 * 64:(h01 + 1) * 64, g, qb * 512:(qb + 1) * 512],
                o_ps[h01][0:DH, :],
            )
        pending.append((g, qb))

    def emit_o(g, qb, kc, o_ps, aw):
        for h01 in range(2):
            if kc == 0:
                o_ps[h01] = psum_o.tile([P, 512], F32, tag="ops",
                                        name=f"o_{g}_{qb}_{h01}")
            h = 2 * g + h01
            nc.tensor.matmul(
                o_ps[h01][0:DH + 1, :],
                v_sb[:, kc, h, :],
                aw[:, h01 * 512:(h01 + 1) * 512],
                start=(kc == 0), stop=(kc == TK - 1),
            )

    def final_flush():
        # normalize the last unit without the DRAM broadcast roundtrip:
        # reciprocal on the [1,512] denominators, then a K=1 matmul
        # broadcast into a psum tile (latency-critical tail path)
        gg, gqb = pending.pop(0)
        assert not pending
        bc_ps = psum_univ.tile([P, 512], F32, tag="u512", name="bc_fin")
        for h01 in range(2):
            d_sb = d_store[(gg, gqb, h01)]
            nc.vector.reciprocal_approx_fast(d_sb, d_sb)
            nc.tensor.matmul(
                bc_ps[h01 * 64:(h01 + 1) * 64, :],
                ones_f32[:, h01 * 64:(h01 + 1) * 64], d_sb,
                start=True, stop=True,
            )
        nc.vector.tensor_mul(
            oT[:, gg, gqb * 512:(gqb + 1) * 512],
            oT[:, gg, gqb * 512:(gqb + 1) * 512],
            bc_ps,
        )

    # ---------------- main schedule ----------------
    # Startup chase: pair-0 q/k and early-v items are emitted between
    # transpose chunks as their token slabs become available, so the
    # S/exp stream starts as soon as slab 0 and qT/kT block 0 exist.
    qk0 = gen_pair_qk(0)
    v0 = gen_v_half(0)
    vh1 = gen_v_half(1)     # needed from attention(4); drained in (1)-(3)

    def run(items):
        for w in items:
            w()

    for r in range(0, 4):
        transpose_chunk(r)
    run(qk0[0:4])            # q load + j0
    run([qk0[7]] + qk0[8:11])  # k load + nj0
    for r in range(4, 8):
        transpose_chunk(r)
    run(qk0[4:7])            # q j1
    run(qk0[11:14])          # k nj1
    for r in range(8, 12):
        transpose_chunk(r)
    run(qk0[14:17])          # k nj2
    run(v0[0:10])            # wv loads + v t0,t1
    for r in range(12, 16):
        transpose_chunk(r)
    run(qk0[17:20])          # k nj3
    run(v0[10:22])           # v t2..t5

    for g in range(NG):
        if g == 0:
            sched = merge(spread(v0[22:], 0, 14),
                          spread(gen_pair_qk(1), 14, N_IT))
        elif g == 1:
            sched = spread(gen_pair_qk(2) + vh1[:18], 0, N_IT)
        elif g == 2:
            sched = spread(gen_pair_qk(3) + vh1[18:36], 0, N_IT)
        elif g == 3:
            sched = spread(gen_pair_qk(4) + vh1[36:], 0, N_IT)
        elif g == 4:
            sched = spread(gen_pair_qk(5) + gen_wout_load(), 0, N_IT)
        elif g < NG - 1:
            sched = spread(gen_pair_qk(g + 1), 0, N_IT)
        else:
            # last pair: overlap the first-half output projection once the
            # qb1 kh==4 normalize flush (iteration 24) has covered qb0
            sched = spread(gen_outproj(0, NQ // P // 2), 24, N_IT)
        attention(g, sched)
    for w in tail_work:
        w()
    tail_work.clear()
    final_flush()
    for w in gen_outproj(NQ // P // 2, NQ // P):
        w()


def build():
    nc = bacc.Bacc("TRN2", target_bir_lowering=False, debug=False,
                   num_devices=NCORES)
    x_ext = nc.dram_tensor("x", [N, D], F32, kind="ExternalInput").ap()
    wqkv_ext = nc.dram_tensor("w_qkv", [D, 3 * INNER], F32, kind="ExternalInput").ap()
    wout_ext = nc.dram_tensor("w_out", [INNER, D], F32, kind="ExternalInput").ap()
    bout_ext = nc.dram_tensor("b_out", [1, D], F32, kind="ExternalInput").ap()
    out_ext = nc.dram_tensor("out", [NQ, D], F32, kind="ExternalOutput").ap()

    with tile.TileContext(nc) as tc:
        with ExitStack() as ctx:
            _build_body(ctx, tc, out_ext, x_ext, wqkv_ext, wout_ext, bout_ext)
    nc.compile()
    return nc


_NC_CACHE = None


def _get_nc():
    global _NC_CACHE
    if _NC_CACHE is None:
        _NC_CACHE = build()
    return _NC_CACHE


def make_in_maps(x, W_qkv, W_out, b_out):
    x = np.ascontiguousarray(np.asarray(x, dtype=np.float32))
    W_qkv = np.ascontiguousarray(np.asarray(W_qkv, dtype=np.float32))
    W_out = np.ascontiguousarray(np.asarray(W_out, dtype=np.float32))
    b_out = np.ascontiguousarray(np.asarray(b_out, dtype=np.float32)).reshape(1, D)
    in_maps = []
    for core in range(NCORES):
        bi, s = core // 2, core % 2
        xb = x[bi]
        if s == 1:  # rotate so this core's queries are rows 0:NQ
            xb = np.concatenate([xb[NQ:], xb[:NQ]], axis=0)
        in_maps.append({
            "x": np.ascontiguousarray(xb),
            "w_qkv": W_qkv,
            "w_out": W_out,
            "b_out": b_out,
        })
    return in_maps


def assemble(outs):
    full = np.empty((B, N, D), np.float32)
    for core in range(NCORES):
        bi, s = core // 2, core % 2
        full[bi, s * NQ:(s + 1) * NQ] = outs[core]
    return full


def kernel(x, W_qkv, W_out, b_out):
    from concourse.bass_utils import run_bass_kernel_spmd

    nc = _get_nc()
    in_maps = make_in_maps(x, W_qkv, W_out, b_out)
    res = run_bass_kernel_spmd(nc, in_maps, core_ids=list(range(NCORES)))
    return assemble([r["out"] for r in res.results])



# revision 11
# speedup vs baseline: 1.0437x; 1.0437x over previous
"""Distributed multi-head attention kernel for 8 Trainium2 NeuronCores.

Problem: x[4,2048,1024] -> qkv proj -> 16-head attention (add_zero_attn)
         -> out proj + bias -> [4,2048,1024]

Sharding: 8 cores = 4 batches x 2 query-halves. Each core computes the
full K/V for its batch (KV projection duplicated across the pair, ~4GFLOP,
far cheaper than any 2-rank collective on this fabric) and attention +
output projection for its own 1024 queries. Zero collectives; host
reassembles by concatenation only.

add_zero_attn appends a zero key & value token: the value row is zero so it
only adds +1 to each softmax denominator. We therefore never materialize it;
denominators come from a ones-column appended to V (column 64 of each head's
v tile) and get +1 before the reciprocal.

Schedule notes:
- x^T is produced by the DMA XBAR transpose (cast f32->bf16 on VectorE,
  then dma_start_transpose SBUF->SBUF), so the PE does no transposes and
  the S/exp stream starts as soon as token slab 0 and the first q/k weight
  blocks exist (~8us instead of ~55us).
- The attention inner loop leaves PE gaps while ScalarE streams exp, so
  all projection work is chopped into small quanta and drained inside the
  attention iterations.
- Output projection is split: partial sums over inner chunks c=0..5 (+bias)
  are computed during pairs 6/7 and parked in DRAM (psum -> DRAM DMA);
  the final c=6,7 matmuls + a VectorE add of the partial close each token
  group as soon as its last oT chunk is normalized. This leaves only the
  last query-half's final pass (8 small groups) as tail.
"""

import sys

sys.path.insert(0, "/opt/trn_rl_repo")

from contextlib import ExitStack

import numpy as np

import concourse.bass as bass
import concourse.tile as tile
from concourse import bacc, mybir

P = 128
B, N, D = 4, 2048, 1024
H, DH = 16, 64
INNER = H * DH  # 1024
SCALE = DH ** -0.5
NQ = N // 2     # queries per core
NCORES = 8

F32 = mybir.dt.float32
BF16 = mybir.dt.bfloat16

DC = D // P         # 8 chunks of the model dim
IC = INNER // P     # 8 chunks of the inner dim
TK = N // P         # 16 kpos chunks
NG = H // 2         # 8 head pairs
QB = NQ // 512      # 2 query blocks of 512


def _build_body(ctx: ExitStack, tc, out_ext, x_ext, wqkv_ext, wout_ext, bout_ext):
    nc = tc.nc

    dram_pool = ctx.enter_context(tc.tile_pool(name="dram", bufs=1, space="DRAM"))
    ddram = dram_pool.tile([NG, QB, 2, 512], F32, tag="ddram")
    # parked output-projection partials: [qb, t, fh] -> [128, 512]
    pdram = dram_pool.tile([QB, 4, 2, P, 512], F32, tag="pdram")

    consts = ctx.enter_context(tc.tile_pool(name="consts", bufs=1))
    xstage = ctx.enter_context(tc.tile_pool(name="xstage", bufs=2))
    xbfst = ctx.enter_context(tc.tile_pool(name="xbfst", bufs=2))
    wstage = ctx.enter_context(tc.tile_pool(name="wstage", bufs=2))
    wqk_pool = ctx.enter_context(tc.tile_pool(name="wqk", bufs=3))
    wv_pool = ctx.enter_context(tc.tile_pool(name="wv", bufs=1))
    persist = ctx.enter_context(tc.tile_pool(name="persist", bufs=1))
    aw_pool = ctx.enter_context(tc.tile_pool(name="attnw", bufs=3))
    d_pool = ctx.enter_context(tc.tile_pool(name="dsb", bufs=2))
    bcs_pool = ctx.enter_context(tc.tile_pool(name="bcsb", bufs=1))
    outst = ctx.enter_context(tc.tile_pool(name="outst", bufs=2))
    landing = ctx.enter_context(tc.tile_pool(name="landing", bufs=2))

    # psum: 2x[128,1024] scores (banks 0-3), 2x[128,512] o-accum (4-5),
    # 2x[128,512] universal for proj/outproj (6-7)
    psum_s = ctx.enter_context(tc.tile_pool(name="psum_s", bufs=2, space="PSUM"))
    psum_o = ctx.enter_context(tc.tile_pool(name="psum_o", bufs=2, space="PSUM"))
    psum_univ = ctx.enter_context(tc.tile_pool(name="psum_univ", bufs=2, space="PSUM"))

    # ---- persistent SBUF arrays ----
    # xT split into 4 independent 512-token slabs (DMA-transpose writes one
    # 128-token column block at a time; consumers read within one slab)
    xT_slabs = [persist.tile([P, DC, 512], BF16, tag=f"xT{s}", name=f"xT_{s}")
                for s in range(4)]

    def xT(c, lo, width):
        s, off = lo // 512, lo % 512
        assert off + width <= 512
        return xT_slabs[s][:, c, off:off + width]

    qT = persist.tile([P, IC, NQ], BF16, tag="qT")
    kT = persist.tile([P, IC, N], BF16, tag="kT")
    v_sb = persist.tile([P, TK, H, DH + 1], BF16, tag="v")   # col DH is ones
    oT = persist.tile([P, IC, NQ], BF16, tag="oT")
    wout = persist.tile([P, IC, D], BF16, tag="wout")

    ones_lhsT = consts.tile([1, P], BF16, tag="ones")
    nc.vector.memset(ones_lhsT, 1.0)
    ones_f32 = consts.tile([1, P], F32, tag="onesf")
    nc.vector.memset(ones_f32, 1.0)
    bout_bf = consts.tile([1, D], BF16, tag="bout")
    bout_f32 = consts.tile([1, D], F32, tag="boutf")
    nc.gpsimd.dma_start(bout_f32, bout_ext)
    nc.vector.tensor_copy(bout_bf, bout_f32)

    # ones column of v (written once; v evictions fill the rest)
    for t in range(TK):
        nc.vector.memset(v_sb[:, t, :, DH:DH + 1], 1.0)

    # ---- x pipeline: DMA load f32 chunk -> VectorE cast bf16 ->
    # DMA XBAR transpose (scalar queue) into the xT slab. No PE involved.
    xbf_hold = {}

    def chunk_load(r):
        x_f = xstage.tile([P, D], F32, tag="xf", name=f"xf_{r}")
        nc.sync.dma_start(x_f, x_ext[r * P:(r + 1) * P, :])
        xbf_hold[("f", r)] = x_f

    def chunk_cast(r):
        x_b = xbfst.tile([P, D], BF16, tag="xb", name=f"xb_{r}")
        nc.vector.tensor_copy(x_b, xbf_hold.pop(("f", r)))
        xbf_hold[("b", r)] = x_b
        if r + 3 < 16:
            chunk_load(r + 3)

    def chunk_tp(r):
        s, off = r // 4, (r % 4) * P
        nc.scalar.dma_start_transpose(
            xT_slabs[s][:, :, off:off + P], xbf_hold.pop(("b", r)))

    wqkv_view = wqkv_ext.rearrange("(c p) f -> p c f", p=P)   # [128, DC, 3072]
    wout_view = wout_ext.rearrange("(c p) f -> p c f", p=P)   # [128, IC, 1024]

    # ---------------- work-quantum builders ----------------
    # Each quantum is a closure doing ~400-900ns of engine work. They are
    # drained a few per attention iteration so the PE never idles while
    # ScalarE streams exp, and ScalarE starts as early as possible.

    def gen_pair_qk(g):
        """Work items computing qT[:,g] and kT[:,g]."""
        items = []
        holder = {}

        def load_w(m, key):
            def run():
                w_f = wstage.tile([P, DC, P], F32, tag="wf", name=f"wf_{key}")
                nc.gpsimd.dma_start(w_f, wqkv_view[:, :, m * P:(m + 1) * P])
                w_b = wqk_pool.tile([P, DC, P], BF16, tag="wqk", name=f"wb_{key}")
                nc.vector.tensor_copy(w_b, w_f)
                holder[key] = w_b
            return run

        # qT column block g: two 512-query halves
        items.append(load_w(g, f"q{g}"))
        for j in range(QB):
            def qa(j=j):
                ps = psum_univ.tile([P, 512], F32, tag="u512", name=f"pq_{g}_{j}")
                holder[f"pq{j}"] = ps
                for c in range(4):
                    nc.tensor.matmul(ps, holder[f"q{g}"][:, c, :],
                                     xT(c, j * 512, 512),
                                     start=(c == 0), stop=False)
            def qb_(j=j):
                ps = holder[f"pq{j}"]
                for c in range(4, DC):
                    nc.tensor.matmul(ps, holder[f"q{g}"][:, c, :],
                                     xT(c, j * 512, 512),
                                     start=False, stop=(c == DC - 1))
            def qe(j=j):
                nc.vector.tensor_copy(qT[:, g, j * 512:(j + 1) * 512],
                                      holder[f"pq{j}"])
            items += [qa, qb_, qe]

        # kT column block g: four 512-token blocks
        items.append(load_w(8 + g, f"k{g}"))
        for nj in range(4):
            def ka(nj=nj):
                ps = psum_univ.tile([P, 512], F32, tag="u512", name=f"pk_{g}_{nj}")
                holder[f"pk{nj}"] = ps
                for c in range(4):
                    nc.tensor.matmul(ps, holder[f"k{g}"][:, c, :],
                                     xT(c, nj * 512, 512),
                                     start=(c == 0), stop=False)
            def kb(nj=nj):
                ps = holder[f"pk{nj}"]
                for c in range(4, DC):
                    nc.tensor.matmul(ps, holder[f"k{g}"][:, c, :],
                                     xT(c, nj * 512, 512),
                                     start=False, stop=(c == DC - 1))
            def ke(nj=nj):
                nc.vector.tensor_copy(kT[:, g, nj * 512:(nj + 1) * 512],
                                      holder[f"pk{nj}"])
            items += [ka, kb, ke]
        return items

    def gen_v_half(nh):
        """Work items computing v for heads nh*8 .. nh*8+7 (pairs 4nh..4nh+3).

        v needs kpos on partitions, so x^T chunks are the stationary operand
        and the 8-head weight slab [128, DC, 512] is the moving one (N=512)."""
        items = []
        holder = {}

        def load_wv(blk):
            def run():
                if blk == 0:
                    holder["wvh"] = wv_pool.tile([P, DC, 512], BF16, tag="wvh",
                                                 name=f"wvh_{nh}")
                m = 16 + nh * 4 + blk
                w_f = wstage.tile([P, DC, P], F32, tag="wf", name=f"wf_v_{m}")
                nc.gpsimd.dma_start(w_f, wqkv_view[:, :, m * P:(m + 1) * P])
                nc.vector.tensor_copy(
                    holder["wvh"][:, :, blk * P:(blk + 1) * P], w_f)
            return run

        for blk in range(4):
            items.append(load_wv(blk))
        for t in range(TK):
            def va(t=t):
                ps = psum_univ.tile([P, 512], F32, tag="u512",
                                    name=f"pv_{t}_{nh}")
                holder[f"pv{t}"] = ps
                for c in range(4):
                    nc.tensor.matmul(ps, xT(c, t * P, P),
                                     holder["wvh"][:, c, :],
                                     start=(c == 0), stop=False)
            def vb(t=t):
                ps = holder[f"pv{t}"]
                for c in range(4, DC):
                    nc.tensor.matmul(ps, xT(c, t * P, P),
                                     holder["wvh"][:, c, :],
                                     start=False, stop=(c == DC - 1))
            def ve(t=t):
                nc.vector.tensor_copy(
                    v_sb[:, t, nh * 8:(nh + 1) * 8, 0:DH],
                    holder[f"pv{t}"].rearrange("p (h d) -> p h d", h=8),
                )
            items += [va, vb, ve]
        return items

    def gen_wout_load():
        items = []
        for m in range(DC):
            def run(m=m):
                w_f = wstage.tile([P, IC, P], F32, tag="wf", name=f"wf_o_{m}")
                nc.gpsimd.dma_start(w_f, wout_view[:, :, m * P:(m + 1) * P])
                nc.vector.tensor_copy(wout[:, :, m * P:(m + 1) * P], w_f)
            items.append(run)
        return items

    # ---- output projection, split into parked partials + finals ----
    # token chunks: qb0 -> t 0..3, qb1 -> t 4..7 (absolute chunk index)
    def gen_outproj_partial(qb):
        """Partial out-proj (c = 0..5 and bias) for query block qb; the psum
        is parked to DRAM. Needs oT chunks 0..5 of this qb normalized."""
        items = []
        holder = {}
        for ti in range(4):
            t = qb * 4 + ti
            for fh in range(2):
                def oa(t=t, ti=ti, fh=fh):
                    ps = psum_univ.tile([P, 512], F32, tag="u512",
                                        name=f"pop_{t}_{fh}")
                    holder[(t, fh)] = ps
                    for c in range(4):
                        nc.tensor.matmul(ps, oT[:, c, t * P:(t + 1) * P],
                                         wout[:, c, fh * 512:(fh + 1) * 512],
                                         start=(c == 0), stop=False)
                def ob(t=t, ti=ti, fh=fh):
                    ps = holder[(t, fh)]
                    for c in range(4, 6):
                        nc.tensor.matmul(ps, oT[:, c, t * P:(t + 1) * P],
                                         wout[:, c, fh * 512:(fh + 1) * 512],
                                         start=False, stop=False)
                    nc.tensor.matmul(ps, ones_lhsT,
                                     bout_bf[:, fh * 512:(fh + 1) * 512],
                                     start=False, stop=True)
                def opark(qb=qb, t=t, ti=ti, fh=fh):
                    o_par = outst.tile([P, 512], F32, tag="osb",
                                       name=f"opar_{t}_{fh}")
                    nc.vector.tensor_copy(o_par, holder.pop((t, fh)))
                    nc.sync.dma_start(pdram[qb, ti, fh], o_par)
                items += [oa, ob, opark]
        return items

    def gen_outproj_final(qb):
        """Final out-proj for query block qb: c=6,7 matmuls + add the parked
        partial. Needs oT chunks 6,7 of this qb normalized (chunk 7 is the
        last pair)."""
        items = []
        holder = {}
        for ti in range(4):
            t = qb * 4 + ti
            for fh in range(2):
                def pref(qb=qb, ti=ti, fh=fh):
                    ld = landing.tile([P, 512], F32, tag="land",
                                      name=f"ld_{qb}_{ti}_{fh}")
                    nc.sync.dma_start(ld, pdram[qb, ti, fh])
                    holder[("ld", ti, fh)] = ld
                def fin(t=t, ti=ti, fh=fh):
                    ps = psum_univ.tile([P, 512], F32, tag="u512",
                                        name=f"pof_{t}_{fh}")
                    holder[("ps", ti, fh)] = ps
                    for c in range(6, IC):
                        nc.tensor.matmul(ps, oT[:, c, t * P:(t + 1) * P],
                                         wout[:, c, fh * 512:(fh + 1) * 512],
                                         start=(c == 6), stop=(c == IC - 1))
                def emit(t=t, ti=ti, fh=fh):
                    o_sb = outst.tile([P, 512], F32, tag="osb",
                                      name=f"osb_{t}_{fh}")
                    nc.vector.tensor_add(o_sb, holder.pop(("ps", ti, fh)),
                                         holder.pop(("ld", ti, fh)))
                    nc.gpsimd.dma_start(
                        out_ext[t * P:(t + 1) * P, fh * 512:(fh + 1) * 512],
                        o_sb)
                items.append((pref, fin, emit))
        # interleave so each group's landing prefetch runs 2 groups ahead
        out = [items[0][0], items[1][0]]
        for k, (pref, fin, emit) in enumerate(items):
            out.append(fin)
            if k + 2 < len(items):
                out.append(items[k + 2][0])
            out.append(emit)
        return out

    # ---------------- attention ----------------
    pending = []
    d_store = {}

    def flush_normalize():
        # drain all finished groups: broadcast 1/(rowsum+1) over the 64
        # partitions of each head and normalize oT in place
        while pending:
            gg, gqb = pending.pop(0)
            bc = bcs_pool.tile([P, 512], F32, tag="bcs", name=f"bc_{gg}_{gqb}")
            for h01 in range(2):
                a = ddram[gg, gqb, h01]
                bcast_src = bass.AP(tensor=a.tensor, offset=a.offset,
                                    ap=[[0, 64]] + list(a.ap))
                nc.gpsimd.dma_start(bc[h01 * 64:(h01 + 1) * 64, :], bcast_src)
            nc.vector.reciprocal_approx_fast(bc, bc)
            nc.vector.tensor_mul(
                oT[:, gg, gqb * 512:(gqb + 1) * 512],
                oT[:, gg, gqb * 512:(gqb + 1) * 512],
                bc,
            )

    N_IT = QB * TK

    def spread(items, it_lo, it_hi):
        """Distribute work items evenly across iterations [it_lo, it_hi)."""
        sched = [[] for _ in range(N_IT)]
        span = it_hi - it_lo
        for i in range(span):
            lo = len(items) * i // span
            hi = len(items) * (i + 1) // span
            sched[it_lo + i].extend(items[lo:hi])
        return sched

    def merge(*scheds):
        out = [[] for _ in range(N_IT)]
        for s in scheds:
            for i, lst in enumerate(s):
                out[i].extend(lst)
        return out

    # o-matmuls for the last two kpos chunks of a unit (and its eviction)
    # are deferred into the next unit's first iterations so the next unit's
    # S/exp stream starts without an o-tail bubble on ScalarE
    tail_work = []

    def attention(g, sched):
        """Attention for head pair g; drains sched[it] along the way."""

        def drain(it):
            for w in sched[it]:
                w()

        for qb in range(QB):
            o_ps = [None, None]
            aw_ring = {}
            for kc in range(TK):
                if kc == 8:
                    flush_normalize()
                st = psum_s.tile([P, 1024], F32, tag="st",
                                 name=f"st_{g}_{qb}_{kc}")
                for h01 in range(2):
                    lo = h01 * 64
                    nc.tensor.matmul(
                        st[:, h01 * 512:(h01 + 1) * 512],
                        kT[lo:lo + 64, g, kc * P:(kc + 1) * P],
                        qT[lo:lo + 64, g, qb * 512:(qb + 1) * 512],
                        start=True, stop=True,
                    )
                aw = aw_pool.tile([P, 1024], BF16, tag="aw",
                                  name=f"aw_{g}_{qb}_{kc}")
                nc.scalar.activation(
                    aw, st, mybir.ActivationFunctionType.Exp, scale=SCALE,
                )
                aw_ring[kc] = aw
                drain(qb * TK + kc)
                if kc == 0 and tail_work:
                    tail_work[0]()
                elif kc == 1 and tail_work:
                    tail_work[1]()
                    tail_work.clear()
                if kc >= 2:
                    emit_o(g, qb, kc - 2, o_ps, aw_ring.pop(kc - 2))
            tail_work[:] = [make_tail(g, qb, TK - 2, o_ps, aw_ring.pop(TK - 2),
                                      evict=False),
                            make_tail(g, qb, TK - 1, o_ps, aw_ring.pop(TK - 1),
                                      evict=True)]

    def make_tail(g, qb, kc, o_ps, aw, evict):
        def run():
            emit_o(g, qb, kc, o_ps, aw)
            if evict:
                evict_unit(g, qb, o_ps)
        return run

    def evict_unit(g, qb, o_ps):
        # evict: D rows (+1 for the zero-attn token) and unnormalized oT
        for h01 in range(2):
            d_sb = d_pool.tile([1, 512], F32, tag="dsb",
                               name=f"d_{g}_{qb}_{h01}")
            nc.vector.tensor_scalar_add(d_sb, o_ps[h01][DH:DH + 1, :], 1.0)
            nc.gpsimd.dma_start(ddram[g, qb, h01], d_sb)
            d_store[(g, qb, h01)] = d_sb
            nc.vector.tensor_copy(
                oT[h01 * 64:(h01 + 1) * 64, g, qb * 512:(qb + 1) * 512],
                o_ps[h01][0:DH, :],
            )
        pending.append((g, qb))

    def emit_o(g, qb, kc, o_ps, aw):
        for h01 in range(2):
            if kc == 0:
                o_ps[h01] = psum_o.tile([P, 512], F32, tag="ops",
                                        name=f"o_{g}_{qb}_{h01}")
            h = 2 * g + h01
            nc.tensor.matmul(
                o_ps[h01][0:DH + 1, :],
                v_sb[:, kc, h, :],
                aw[:, h01 * 512:(h01 + 1) * 512],
                start=(kc == 0), stop=(kc == TK - 1),
            )

    def final_flush():
        # normalize the last unit without the DRAM broadcast roundtrip:
        # reciprocal on the [1,512] denominators, then a K=1 matmul
        # broadcast into a psum tile (latency-critical tail path)
        gg, gqb = pending.pop(0)
        assert not pending
        bc_ps = psum_univ.tile([P, 512], F32, tag="u512", name="bc_fin")
        for h01 in range(2):
            d_sb = d_store[(gg, gqb, h01)]
            nc.vector.reciprocal_approx_fast(d_sb, d_sb)
            nc.tensor.matmul(
                bc_ps[h01 * 64:(h01 + 1) * 64, :],
                ones_f32[:, h01 * 64:(h01 + 1) * 64], d_sb,
                start=True, stop=True,
            )
        nc.vector.tensor_mul(
            oT[:, gg, gqb * 512:(gqb + 1) * 512],
            oT[:, gg, gqb * 512:(gqb + 1) * 512],
            bc_ps,
        )

    # ---------------- main schedule ----------------
    # Startup chase: x chunk 0-7 pipelines + pair-0 q/k j0/nj0 + early-v run
    # before attention; chunks 8-15, the rest of qk0/v0 drain inside pair 0.
    qk0 = gen_pair_qk(0)
    v0 = gen_v_half(0)
    vh1 = gen_v_half(1)     # needed from attention(4); drained in (1)-(3)

    def run(items):
        for w in items:
            w()

    # issue early x loads; the rest are issued as casts free their buffers
    for r in range(3):
        chunk_load(r)

    for r in range(0, 4):
        chunk_cast(r)
        chunk_tp(r)
    run(qk0[0:2])            # q load + j0 first half
    for r in range(4, 6):
        chunk_cast(r)
        chunk_tp(r)
    run(qk0[2:4])            # q j0 rest
    run([qk0[7]] + qk0[8:11])  # k load + nj0
    for r in range(6, 8):
        chunk_cast(r)
        chunk_tp(r)
    run(v0[0:4])             # wv loads
    run(v0[4:10])            # v t0, t1

    for g in range(NG):
        if g == 0:
            sched = merge(
                spread([lambda r=r: chunk_cast(r) for r in range(8, 16)], 0, 8),
                spread([lambda r=r: chunk_tp(r) for r in range(8, 16)], 0, 8),
                spread(qk0[11:14], 1, 3),    # k nj1 (S kc=4 reads at it 4)
                spread(qk0[14:17], 5, 7),    # k nj2 (slab 2 tp'd by it 4)
                spread(qk0[17:20], 8, 11),   # k nj3 (slab 3 tp'd by it 8)
                spread(qk0[4:7], 5, 9),      # q j1
                spread(v0[10:28], 2, 10),    # v t2..t7
                spread(v0[28:52], 10, 18),   # v t8..t15
                spread(gen_pair_qk(1), 18, N_IT))
        elif g == 1:
            sched = spread(gen_pair_qk(2) + vh1[:18], 0, N_IT)
        elif g == 2:
            sched = spread(gen_pair_qk(3) + vh1[18:36], 0, N_IT)
        elif g == 3:
            sched = spread(gen_pair_qk(4) + vh1[36:], 0, N_IT)
        elif g == 4:
            sched = spread(gen_pair_qk(5) + gen_wout_load(), 0, N_IT)
        elif g == 5:
            sched = spread(gen_pair_qk(6), 0, N_IT)
        elif g == 6:
            # qb0 oT chunks 0..5 are normalized by end of pair 5
            sched = merge(spread(gen_pair_qk(7), 0, N_IT),
                          spread(gen_outproj_partial(0), 4, 30))
        else:
            # qb1 partials (oT chunks 0..5 of qb1 normalized by end of p6);
            # qb0 finals after the (7,qb0) flush at iteration 24
            sched = merge(spread(gen_outproj_partial(1), 2, 22),
                          spread(gen_outproj_final(0), 24, N_IT))
        attention(g, sched)
    for w in tail_work:
        w()
    tail_work.clear()
    final_flush()
    run(gen_outproj_final(1))


def build():
    nc = bacc.Bacc("TRN2", target_bir_lowering=False, debug=False,
                   num_devices=NCORES)
    x_ext = nc.dram_tensor("x", [N, D], F32, kind="ExternalInput").ap()
    wqkv_ext = nc.dram_tensor("w_qkv", [D, 3 * INNER], F32, kind="ExternalInput").ap()
    wout_ext = nc.dram_tensor("w_out", [INNER, D], F32, kind="ExternalInput").ap()
    bout_ext = nc.dram_tensor("b_out", [1, D], F32, kind="ExternalInput").ap()
    out_ext = nc.dram_tensor("out", [NQ, D], F32, kind="ExternalOutput").ap()

    with tile.TileContext(nc) as tc:
        with ExitStack() as ctx:
            _build_body(ctx, tc, out_ext, x_ext, wqkv_ext, wout_ext, bout_ext)
    nc.compile()
    return nc


_NC_CACHE = None


def _get_nc():
    global _NC_CACHE
    if _NC_CACHE is None:
        _NC_CACHE = build()
    return _NC_CACHE


def make_in_maps(x, W_qkv, W_out, b_out):
    x = np.ascontiguousarray(np.asarray(x, dtype=np.float32))
    W_qkv = np.ascontiguousarray(np.asarray(W_qkv, dtype=np.float32))
    W_out = np.ascontiguousarray(np.asarray(W_out, dtype=np.float32))
    b_out = np.ascontiguousarray(np.asarray(b_out, dtype=np.float32)).reshape(1, D)
    in_maps = []
    for core in range(NCORES):
        bi, s = core // 2, core % 2
        xb = x[bi]
        if s == 1:  # rotate so this core's queries are rows 0:NQ
            xb = np.concatenate([xb[NQ:], xb[:NQ]], axis=0)
        in_maps.append({
            "x": np.ascontiguousarray(xb),
            "w_qkv": W_qkv,
            "w_out": W_out,
            "b_out": b_out,
        })
    return in_maps


def assemble(outs):
    full = np.empty((B, N, D), np.float32)
    for core in range(NCORES):
        bi, s = core // 2, core % 2
        full[bi, s * NQ:(s + 1) * NQ] = outs[core]
    return full


def kernel(x, W_qkv, W_out, b_out):
    from concourse.bass_utils import run_bass_kernel_spmd

    nc = _get_nc()
    in_maps = make_in_maps(x, W_qkv, W_out, b_out)
    res = run_bass_kernel_spmd(nc, in_maps, core_ids=list(range(NCORES)))
    return assemble([r["out"] for r in res.results])
